# revision 1
# baseline (speedup 1.0000x reference)
"""Trainium2 Bass kernel for a dense transformer block (B=2, T=2048, C=1024, H=16).

Sharding: tensor-parallel attention (2 heads/core) + AllToAll + row-parallel
FFN (512 rows/core) across 8 NeuronCores. All matmuls bf16 with fp32 PSUM
accumulation. LayerNorm affine params are folded into the adjacent weight
matrices on the host.
"""

import numpy as np
import ml_dtypes

import concourse.bass as bass
import concourse.bacc as bacc
import concourse.mybir as mybir
import concourse.tile as tile

F32 = mybir.dt.float32
BF16 = mybir.dt.bfloat16
AF = mybir.ActivationFunctionType
ALU = mybir.AluOpType

N_CORES = 8
B, T, C, H, D, FF = 2, 2048, 1024, 16, 64, 4096
R = B * T            # 4096 total rows
RS = R // N_CORES    # 512 rows per core
KT = C // 128        # 8 k-tiles of the embedding dim
SCALE = 1.0 / np.sqrt(C)
LN_EPS = 1e-5


def build_nc():
    nc = bacc.Bacc(None, target_bir_lowering=False, debug=False, num_devices=N_CORES)

    # ---- per-core inputs (host pre-laid-out) ----
    x_bf = nc.dram_tensor("x_bf", [32, 128, C], BF16, kind="ExternalInput").ap()
    x_f32 = nc.dram_tensor("x_f32", [4, 128, C], F32, kind="ExternalInput").ap()
    wq = nc.dram_tensor("wq", [128, KT, 128], BF16, kind="ExternalInput").ap()
    wk = nc.dram_tensor("wk", [128, KT, 128], BF16, kind="ExternalInput").ap()
    wv = nc.dram_tensor("wv", [128, KT, 128], BF16, kind="ExternalInput").ap()
    bqkv = nc.dram_tensor("bqkv", [128, 3], F32, kind="ExternalInput").ap()
    wo = nc.dram_tensor("wo", [128, KT, C], BF16, kind="ExternalInput").ap()
    bo = nc.dram_tensor("bo", [128, 8], F32, kind="ExternalInput").ap()
    w1 = nc.dram_tensor("w1", [128, KT, FF], BF16, kind="ExternalInput").ap()
    b1 = nc.dram_tensor("b1", [128, 32], F32, kind="ExternalInput").ap()
    w2 = nc.dram_tensor("w2", [128, FF // 128, C], BF16, kind="ExternalInput").ap()
    b2 = nc.dram_tensor("b2", [128, 8], F32, kind="ExternalInput").ap()
    masks = nc.dram_tensor("masks", [4, 128, 512], BF16, kind="ExternalInput").ap()
    y = nc.dram_tensor("y", [4, 128, C], F32, kind="ExternalOutput").ap()

    with tile.TileContext(nc) as tc:
        with (
            tc.tile_pool(name="const", bufs=1) as const,
            tc.tile_pool(name="ps_mm", bufs=4, space="PSUM") as ps_mm,
            tc.tile_pool(name="ps_a", bufs=2, space="PSUM") as ps_a,
            tc.tile_pool(name="ps_bc", bufs=2, space="PSUM") as ps_bc,
            tc.tile_pool(name="attn", bufs=1) as attn,
            tc.tile_pool(name="dram", bufs=1, space="DRAM") as dram,
        ):
            # constants
            ones128 = const.tile([128, 1], BF16)
            nc.any.memset(ones128[:], 1.0)
            ones64 = const.tile([1, 64], BF16)
            nc.any.memset(ones64[:], 1.0)
            epst = const.tile([128, 1], F32)
            nc.any.memset(epst[:], LN_EPS)
            mask_sb = const.tile([4, 128, 512], BF16)
            for d in range(4):
                nc.sync.dma_start(mask_sb[d], masks[d])
            wq_sb = const.tile([128, KT, 128], BF16)
            nc.sync.dma_start(wq_sb[:], wq[:])
            wk_sb = const.tile([128, KT, 128], BF16)
            nc.sync.dma_start(wk_sb[:], wk[:])
            wv_sb = const.tile([128, KT, 128], BF16)
            nc.sync.dma_start(wv_sb[:], wv[:])
            bqkv_sb = const.tile([128, 3], F32)
            nc.sync.dma_start(bqkv_sb[:], bqkv[:])
            wo_sb = const.tile([128, KT, C], BF16)
            nc.sync.dma_start(wo_sb[:], wo[:])
            bo_sb = const.tile([128, 8], F32)
            nc.sync.dma_start(bo_sb[:], bo[:])
            b1_sb = const.tile([128, 32], F32)
            nc.sync.dma_start(b1_sb[:], b1[:])
            b2_sb = const.tile([128, 8], F32)
            nc.sync.dma_start(b2_sb[:], b2[:])

            # attention-stage persistent tiles
            qt_sb = attn.tile([128, R], BF16)   # Q^T, feature-major (2 heads stacked)
            kt_sb = attn.tile([128, R], BF16)   # K^T
            vt_sb = attn.tile([128, R], BF16)   # V^T (pre-transpose)
            v_sb = attn.tile([128, 32, 128], BF16)  # V token-major chunks
            at_sb = attn.tile([128, R], BF16)   # normalized attn out (2 heads), feature-major

            # =============== Stage A: LN1 + transpose to feature-major ===============
            with (
                tc.tile_pool(name="lnp", bufs=4) as lnp,
                tc.tile_pool(name="h1tp", bufs=1) as h1tp,
            ):
                h1t = h1tp.tile([128, KT, R], BF16)   # ln1(x)^T feature-major
                for i in range(32):
                    xt = lnp.tile([128, C], BF16)
                    nc.sync.dma_start(xt[:], x_bf[i])
                    stats = lnp.tile([128, 2, 6], F32)
                    xr = xt[:].rearrange("p (s f) -> p s f", f=512)
                    for s in range(2):
                        nc.vector.bn_stats(out=stats[:, s, :], in_=xr[:, s, :])
                    mv = lnp.tile([128, 2], F32)
                    nc.vector.bn_aggr(out=mv[:], in_=stats[:])
                    rstd = lnp.tile([128, 1], F32)
                    nc.scalar.activation(out=rstd[:], in_=mv[:, 1:2], func=AF.Sqrt,
                                         bias=epst[:], scale=1.0)
                    nc.vector.reciprocal(out=rstd[:], in_=rstd[:])
                    hn = lnp.tile([128, C], BF16)
                    nc.vector.tensor_scalar(out=hn[:], in0=xt[:],
                                            scalar1=mv[:, 0:1], scalar2=rstd[:],
                                            op0=ALU.subtract, op1=ALU.mult)
                    for j in range(KT):
                        nc.sync.dma_start(out=h1t[:, j, 128 * i:128 * (i + 1)],
                                          in_=hn[:, 128 * j:128 * (j + 1)],
                                          transpose=True)

                # =============== Stage B: QKV projections ===============
                for w_sb, out_sb, bcol in ((wq_sb, qt_sb, 0), (wk_sb, kt_sb, 1),
                                           (wv_sb, vt_sb, 2)):
                    for n in range(R // 512):
                        ps = ps_mm.tile([128, 512], F32, tag="psmm")
                        for k in range(KT):
                            nc.tensor.matmul(ps[:], lhsT=w_sb[:, k, :],
                                             rhs=h1t[:, k, 512 * n:512 * (n + 1)],
                                             start=(k == 0), stop=(k == KT - 1))
                        nc.scalar.activation(out=out_sb[:, 512 * n:512 * (n + 1)],
                                             in_=ps[:], func=AF.Identity,
                                             bias=bqkv_sb[:, bcol:bcol + 1], scale=1.0)
            # V token-major
            for j in range(32):
                nc.sync.dma_start(out=v_sb[:, j, :],
                                  in_=vt_sb[:, 128 * j:128 * (j + 1)], transpose=True)

            # =============== Stage C: attention (S^T orientation) ===============
            with tc.tile_pool(name="ptp", bufs=36) as ptp:
                for b in range(B):
                    for qc in range(4):
                        q0 = b * T + 512 * qc
                        nkt = 4 * (qc + 1)
                        pts = {}
                        for h in range(2):
                            hp = 64 * h
                            for k in range(nkt):
                                ps = ps_mm.tile([128, 512], F32, tag="psmm")
                                nc.tensor.matmul(
                                    ps[:],
                                    lhsT=kt_sb[hp:hp + 64,
                                               b * T + 128 * k:b * T + 128 * (k + 1)],
                                    rhs=qt_sb[hp:hp + 64, q0:q0 + 512],
                                    start=True, stop=True, tile_position=(hp, 0))
                                pt = ptp.tile([128, 512], BF16, tag="pt")
                                nc.scalar.activation(out=pt[:], in_=ps[:],
                                                     func=AF.Exp, scale=SCALE)
                                if k >= 4 * qc:
                                    nc.vector.tensor_tensor(out=pt[:], in0=pt[:],
                                                            in1=mask_sb[k - 4 * qc],
                                                            op=ALU.mult)
                                pts[(h, k)] = pt
                        for h in range(2):
                            hp = 64 * h
                            pa = ps_a.tile([128, 512], F32, tag="psa")
                            for k in range(nkt):
                                nc.tensor.matmul(pa[0:64, :],
                                                 lhsT=v_sb[:, b * 16 + k, hp:hp + 64],
                                                 rhs=pts[(h, k)][:],
                                                 start=(k == 0), stop=(k == nkt - 1),
                                                 tile_position=(0, 0))
                                nc.tensor.matmul(pa[64:65, :],
                                                 lhsT=ones128[:],
                                                 rhs=pts[(h, k)][:],
                                                 start=(k == 0), stop=(k == nkt - 1),
                                                 tile_position=(0, 64))
                            rec = ptp.tile([1, 512], BF16, tag="rec")
                            with nc.allow_low_precision(reason="softmax denom bf16"):
                                nc.vector.reciprocal(out=rec[:], in_=pa[64:65, :])
                            pb = ps_bc.tile([64, 512], F32, tag="psbc")
                            nc.tensor.matmul(pb[:], lhsT=ones64[:], rhs=rec[:],
                                             start=True, stop=True)
                            anum = ptp.tile([64, 512], BF16, tag="anum")
                            nc.scalar.copy(out=anum[:], in_=pa[0:64, :])
                            nc.vector.tensor_tensor(out=at_sb[hp:hp + 64, q0:q0 + 512],
                                                    in0=anum[:], in1=pb[:], op=ALU.mult)

            # =============== Stage D: AllToAll ===============
            a2a_in = dram.tile([N_CORES, 128, RS], BF16)
            a2a_out = dram.tile([N_CORES, 128, RS], BF16)
            nc.sync.dma_start(out=a2a_in[:].rearrange("s p n -> p s n"),
                              in_=at_sb[:].rearrange("p (s n) -> p s n", n=RS))
            nc.gpsimd.collective_compute(
                "AllToAll", ALU.bypass,
                replica_groups=[list(range(N_CORES))],
                ins=[a2a_in[:].opt()], outs=[a2a_out[:].opt()],
            )

            with tc.tile_pool(name="ef", bufs=1) as ef:
                # =============== Stage E: Wo + residual + LN2 ===============
                attnt = ef.tile([128, KT, RS], BF16)
                nc.sync.dma_start(out=attnt[:],
                                  in_=a2a_out[:].rearrange("s p n -> p s n"))
                o_tok = ef.tile([128, 4, C], BF16)
                for m in range(8):
                    ps = ps_mm.tile([128, 512], F32, tag="psmm")
                    for k in range(KT):
                        nc.tensor.matmul(ps[:], lhsT=wo_sb[:, k, 128 * m:128 * (m + 1)],
                                         rhs=attnt[:, k, :],
                                         start=(k == 0), stop=(k == KT - 1))
                    ot = ef.tile([128, 512], BF16, tag="ot")
                    nc.scalar.activation(out=ot[:], in_=ps[:], func=AF.Identity,
                                         bias=bo_sb[:, m:m + 1], scale=1.0)
                    for j in range(4):
                        nc.sync.dma_start(out=o_tok[:, j, 128 * m:128 * (m + 1)],
                                          in_=ot[:, 128 * j:128 * (j + 1)],
                                          transpose=True)
                x2 = ef.tile([128, 4, C], F32)
                h2t = ef.tile([128, KT, RS], BF16)
                for j in range(4):
                    xs = ef.tile([128, C], F32, tag="xs")
                    nc.sync.dma_start(xs[:], x_f32[j])
                    nc.vector.tensor_tensor(out=x2[:, j, :], in0=xs[:],
                                            in1=o_tok[:, j, :], op=ALU.add)
                    stats2 = ef.tile([128, 2, 6], F32, tag="st2")
                    x2r = x2[:, j, :].rearrange("p (s f) -> p s f", f=512)
                    for s in range(2):
                        nc.vector.bn_stats(out=stats2[:, s, :], in_=x2r[:, s, :])
                    mv2 = ef.tile([128, 2], F32, tag="mv2")
                    nc.vector.bn_aggr(out=mv2[:], in_=stats2[:])
                    rstd2 = ef.tile([128, 1], F32, tag="rstd2")
                    nc.scalar.activation(out=rstd2[:], in_=mv2[:, 1:2], func=AF.Sqrt,
                                         bias=epst[:], scale=1.0)
                    nc.vector.reciprocal(out=rstd2[:], in_=rstd2[:])
                    h2 = ef.tile([128, C], BF16, tag="h2")
                    nc.vector.tensor_scalar(out=h2[:], in0=x2[:, j, :],
                                            scalar1=mv2[:, 0:1], scalar2=rstd2[:],
                                            op0=ALU.subtract, op1=ALU.mult)
                    for k in range(KT):
                        nc.sync.dma_start(out=h2t[:, k, 128 * j:128 * (j + 1)],
                                          in_=h2[:, 128 * k:128 * (k + 1)],
                                          transpose=True)

                # =============== Stage F: FFN ===============
                hid = ef.tile([128, 32, RS], BF16)
                with tc.tile_pool(name="w1p", bufs=3) as w1p:
                    for m in range(32):
                        w1t = w1p.tile([128, KT, 128], BF16, tag="w1t")
                        nc.sync.dma_start(w1t[:], w1[:, :, 128 * m:128 * (m + 1)])
                        ps = ps_mm.tile([128, 512], F32, tag="psmm")
                        for k in range(KT):
                            nc.tensor.matmul(ps[:], lhsT=w1t[:, k, :], rhs=h2t[:, k, :],
                                             start=(k == 0), stop=(k == KT - 1))
                        nc.scalar.activation(out=hid[:, m, :], in_=ps[:], func=AF.Relu,
                                             bias=b1_sb[:, m:m + 1], scale=1.0)
                o2_tok = ef.tile([128, 4, C], BF16)
                with tc.tile_pool(name="w2p", bufs=2) as w2p:
                    for m in range(8):
                        w2t = w2p.tile([128, 32, 128], BF16, tag="w2t")
                        nc.sync.dma_start(w2t[:], w2[:, :, 128 * m:128 * (m + 1)])
                        ps = ps_mm.tile([128, 512], F32, tag="psmm")
                        for k in range(32):
                            nc.tensor.matmul(ps[:], lhsT=w2t[:, k, :], rhs=hid[:, k, :],
                                             start=(k == 0), stop=(k == 31))
                        o2t = ef.tile([128, 512], BF16, tag="o2t")
                        nc.scalar.activation(out=o2t[:], in_=ps[:], func=AF.Identity,
                                             bias=b2_sb[:, m:m + 1], scale=1.0)
                        for j in range(4):
                            nc.sync.dma_start(out=o2_tok[:, j, 128 * m:128 * (m + 1)],
                                              in_=o2t[:, 128 * j:128 * (j + 1)],
                                              transpose=True)
                for j in range(4):
                    yt = ef.tile([128, C], F32, tag="yt")
                    nc.vector.tensor_tensor(out=yt[:], in0=x2[:, j, :],
                                            in1=o2_tok[:, j, :], op=ALU.add)
                    nc.sync.dma_start(y[j], yt[:])

    nc.compile()
    return nc


def prep_inputs(x, Wq, Wk, Wv, Wo, bo, W1, b1, W2, b2, g1, be1, g2, be2):
    """Host-side sharding / layout prep. Returns list of per-core input dicts."""
    bf = ml_dtypes.bfloat16
    x = np.asarray(x, np.float32).reshape(R, C)
    g1 = np.asarray(g1, np.float32); be1 = np.asarray(be1, np.float32)
    g2 = np.asarray(g2, np.float32); be2 = np.asarray(be2, np.float32)
    Wq = np.asarray(Wq, np.float32); Wk = np.asarray(Wk, np.float32)
    Wv = np.asarray(Wv, np.float32); Wo = np.asarray(Wo, np.float32)
    W1 = np.asarray(W1, np.float32); W2 = np.asarray(W2, np.float32)
    bo = np.asarray(bo, np.float32); b1 = np.asarray(b1, np.float32)
    b2 = np.asarray(b2, np.float32)

    Wq_f = g1[:, None] * Wq; bq_f = be1 @ Wq
    Wk_f = g1[:, None] * Wk; bk_f = be1 @ Wk
    Wv_f = g1[:, None] * Wv; bv_f = be1 @ Wv
    W1_f = g2[:, None] * W1; b1_f = b1 + be2 @ W1

    def lhsT_layout(w):  # [C_in, M] -> [128, C_in//128, M]
        ci, m = w.shape
        return np.ascontiguousarray(
            w.reshape(ci // 128, 128, m).transpose(1, 0, 2)).astype(bf)

    def bias_layout(v):  # [M] -> [128, M//128]
        return np.ascontiguousarray(v.reshape(-1, 128).T).astype(np.float32)

    x_bf_full = np.ascontiguousarray(x.reshape(32, 128, C)).astype(bf)
    wo_l = lhsT_layout(Wo)
    w1_l = lhsT_layout(W1_f)
    w2_l = lhsT_layout(W2)
    bo_l = bias_layout(bo)
    b1_l = bias_layout(b1_f)
    b2_l = bias_layout(b2)

    # causal partial-tile masks: mask[d][kl, ql] = 1 if 128*d + kl <= ql
    masks = np.zeros((4, 128, 512), np.float32)
    for d in range(4):
        kl = 128 * d + np.arange(128)[:, None]
        ql = np.arange(512)[None, :]
        masks[d] = (kl <= ql).astype(np.float32)
    masks = masks.astype(bf)

    ins = []
    for c in range(N_CORES):
        cs = slice(128 * c, 128 * (c + 1))
        ins.append({
            "x_bf": x_bf_full,
            "x_f32": np.ascontiguousarray(
                x[RS * c:RS * (c + 1)].reshape(4, 128, C)).astype(np.float32),
            "wq": lhsT_layout(Wq_f[:, cs]),
            "wk": lhsT_layout(Wk_f[:, cs]),
            "wv": lhsT_layout(Wv_f[:, cs]),
            "bqkv": np.ascontiguousarray(
                np.stack([bq_f[cs], bk_f[cs], bv_f[cs]], axis=1)).astype(np.float32),
            "wo": wo_l, "bo": bo_l,
            "w1": w1_l, "b1": b1_l,
            "w2": w2_l, "b2": b2_l,
            "masks": masks,
        })
    return ins


_NC_CACHE = {}


def kernel(**inputs):
    from concourse.bass_utils import run_bass_kernel_spmd
    if "nc" not in _NC_CACHE:
        _NC_CACHE["nc"] = build_nc()
    nc = _NC_CACHE["nc"]
    ins = prep_inputs(**inputs)
    res = run_bass_kernel_spmd(nc, ins, core_ids=list(range(N_CORES)))
    out = np.concatenate([r["y"].reshape(RS, C) for r in res.results], axis=0)
    return out.reshape(B, T, C).astype(np.float32)
